# revision 16
# baseline (speedup 1.0000x reference)
"""Trainium2 Bass kernel for nn_DensityVQC (batched 2-qubit VQC Z-expectation).

Algebra
-------
The reference builds rho_b = conj(psi_b) psi_b^T (note: transpose of the
standard density matrix), evolves rho' = U rho U^dag and returns
tr(rho' Z0) with Z0 = diag(1,1,-1,-1).  This collapses to a per-row
quadratic form: with V = conj(U) (the transposed-rho convention flips the
conjugation) and phi = V psi,

    out_b = |phi_0|^2 + |phi_1|^2 - |phi_2|^2 - |phi_3|^2
          = 2 * || C psi_b ||^2 - ||psi_b||^2        (C = V[0:2, :], U unitary)
          = || A r_b + B m_b ||^2 - 1                (inputs are unit-norm)

with real 4x4 matrices A = sqrt(2)*[Re C; Im C], B = sqrt(2)*[-Im C; Re C].
So the device kernel is: per batch row (r, m in R^4), compute w = A r + B m,
then out = sum(w^2) - 1.  No [B,4,4] density matrices are ever materialized.

Device mapping (per core, pure data parallel over 8 cores)
----------------------------------------------------------
Host-side marshalling (the sharding step) reshapes each core's slice into
component-major layout [128 = 32 groups x 4 comps, 4096] so the device
needs no transposes; loads are perfectly contiguous plain DMAs.

Per supertile of 512 free columns (16384 batch rows):
  1. PE: phi = blkdiag32(A^T)^T . rt + blkdiag32(B^T)^T . mt  (two
     accumulating float32r matmuls at full PE rate, moving operands are
     DMA-resident input slices)
  2. ACT Square: S = phi^2 -> SBUF (f32r)
  3. PE: one reduce matmul (stationary = group-sum pattern [128,32],
     moving = S) -> out32 [32, 512] in PSUM
  4. ACT/DVE copy with -1 bias -> resident [32, 4096] output tile
A dummy-matmul burst during the load window warms the PE HAM clock-gate so
the real matmuls run at 2.4 GHz.  The host un-permutes the [32, 4096]
output tile back to batch order (pure data marshalling).
"""

import sys
import numpy as np

if "/opt/trn_rl_repo" not in sys.path:
    sys.path.insert(0, "/opt/trn_rl_repo")

import concourse.bass as bass
import concourse.tile as tile
from concourse import bacc, mybir
from concourse import bass_utils

N_CORES = 8
BSZ = 1_048_576
BC = BSZ // N_CORES            # 131072 rows per core
NCOL = BC // 32                # 4096 component-major free columns
N_ST = NCOL // 512             # 8 supertiles
N_WARM = 16                    # HAM warm-up matmuls (~3.8us of PE busy)
F32 = mybir.dt.float32
F32R = mybir.dt.float32r
N_LAYERS = 6


def _circuit_unitary(ry, rz):
    """4x4 circuit unitary, float64 mirror of reference._circuit_unitary."""
    ry = np.asarray(ry, dtype=np.float64)
    rz = np.asarray(rz, dtype=np.float64)
    cnot = np.array(
        [[1, 0, 0, 0], [0, 1, 0, 0], [0, 0, 0, 1], [0, 0, 1, 0]],
        dtype=np.complex128,
    )

    def _ry(th):
        c, s = np.cos(th / 2), np.sin(th / 2)
        return np.array([[c, -s], [s, c]], dtype=np.complex128)

    def _rz(th):
        return np.diag([np.exp(-0.5j * th), np.exp(0.5j * th)])

    u = np.eye(4, dtype=np.complex128)
    for l in range(ry.shape[0]):
        ry_full = np.kron(_ry(ry[l, 0]), _ry(ry[l, 1]))
        rz_full = np.kron(_rz(rz[l, 0]), _rz(rz[l, 1]))
        u = cnot @ (rz_full @ (ry_full @ u))
    return u


def _host_consts(ry_params, rz_params):
    u = _circuit_unitary(ry_params, rz_params)
    c = np.conj(u)[0:2, :]
    a = np.sqrt(2.0) * np.vstack([c.real, c.imag])     # 4x4, w = A r + B m
    b = np.sqrt(2.0) * np.vstack([-c.imag, c.real])
    eye32 = np.eye(32, dtype=np.float32)
    # lhsT[k=4g+c, m=4g+j] = A[j, c]  ->  block_diag of A.T
    ablk = np.kron(eye32, a.T.astype(np.float32)).astype(np.float32)
    bblk = np.kron(eye32, b.T.astype(np.float32)).astype(np.float32)
    zsum = np.kron(eye32, np.ones((4, 1), dtype=np.float32)).astype(np.float32)
    return ablk, bblk, zsum


# Any fixed permutation of the 4096 32-row blocks works (the host inverts
# it); identity keeps the input marshalling a pure reshape+transpose.
def _to_component_major(x):
    """x [BC,4] f32 -> [128, NCOL] f32: column N holds batch rows
    [32N, 32N+32) x 4 comps on the 128 partitions."""
    return np.ascontiguousarray(x.reshape(NCOL, 128).T)


def _from_out32(y):
    """y [32, NCOL] -> [BC]: batch b = 32N + g  ->  y[g, N]."""
    return np.ascontiguousarray(y.T).reshape(-1)


def _build_program():
    nc = bacc.Bacc("TRN2", target_bir_lowering=False, debug=False)
    rt_d = nc.dram_tensor("rt", [128, NCOL], F32R, kind="ExternalInput")
    mt_d = nc.dram_tensor("mt", [128, NCOL], F32R, kind="ExternalInput")
    cst_d = nc.dram_tensor("cst", [128, 288], F32R, kind="ExternalInput")
    out_d = nc.dram_tensor("out", [32, NCOL], F32, kind="ExternalOutput")

    with tile.TileContext(nc) as tc:
        with (
            tc.tile_pool(name="const", bufs=1) as cpool,
            tc.tile_pool(name="io", bufs=1) as iopool,
            tc.tile_pool(name="work", bufs=4) as wpool,
            tc.tile_pool(name="psum", bufs=3, space=bass.MemorySpace.PSUM) as ppool,
        ):
            cst = cpool.tile([128, 288], F32R, name="cst_t")
            nc.scalar.dma_start(cst[:], cst_d.ap())
            ablk = cst[:, 0:128]
            bblk = cst[:, 128:256]
            zsum = cst[:, 256:288]

            rt_t = iopool.tile([128, NCOL], F32R, name="rt_t")
            mt_t = iopool.tile([128, NCOL], F32R, name="mt_t")
            out_full = iopool.tile([32, NCOL], F32, name="out_full")

            half = NCOL // 2
            for h in range(2):
                hs = bass.ts(h, half)
                nc.sync.dma_start(rt_t[:, hs], rt_d.ap()[:, hs])
                nc.scalar.dma_start(mt_t[:, hs], mt_d.ap()[:, hs])

            # HAM warm-up: dense dummy matmuls on the const tile keep the PE
            # busy through the load window so real matmuls run at 2.4 GHz.
            warm = ppool.tile([128, 288], F32, name="warm", bufs=1)
            for _ in range(N_WARM):
                nc.tensor.matmul(warm[:], ablk, cst[:])

            for st in range(N_ST):
                cs = bass.ts(st, 512)
                phi = ppool.tile([128, 512], F32, name="phi", bufs=4)
                nc.tensor.matmul(
                    phi[:], ablk, rt_t[:, cs], start=True, stop=False
                )
                nc.tensor.matmul(
                    phi[:], bblk, mt_t[:, cs], start=False, stop=True
                )

                s_sb = wpool.tile([128, 512], F32R, name="s_sb")
                nc.scalar.activation(
                    s_sb[:], phi[:], mybir.ActivationFunctionType.Square
                )

                out32 = ppool.tile([32, 512], F32, name="out32")
                nc.tensor.matmul(out32[:], zsum, s_sb[:])

                # PSUM -> SBUF with the -1 fold; alternate engines.
                if st % 2 == 0:
                    nc.vector.tensor_scalar_add(out_full[:, cs], out32[:], -1.0)
                else:
                    nc.scalar.activation(
                        out_full[:, cs],
                        out32[:],
                        mybir.ActivationFunctionType.Copy,
                        bias=-1.0,
                    )

                if st == 3:
                    nc.gpsimd.dma_start(
                        out_d.ap()[:, 0 : NCOL // 2], out_full[:, 0 : NCOL // 2]
                    )
            nc.gpsimd.dma_start(
                out_d.ap()[:, NCOL // 2 : NCOL], out_full[:, NCOL // 2 : NCOL]
            )
    nc.compile()
    return nc


_PROG_CACHE = None


def _get_program():
    global _PROG_CACHE
    if _PROG_CACHE is None:
        _PROG_CACHE = _build_program()
    return _PROG_CACHE


def _run(ry_params, rz_params, states_real, states_imag, **hw_kwargs):
    ablk, bblk, zsum = _host_consts(ry_params, rz_params)
    cst = np.concatenate([ablk, bblk, zsum], axis=1).astype(np.float32)
    states_real = np.ascontiguousarray(states_real, dtype=np.float32)
    states_imag = np.ascontiguousarray(states_imag, dtype=np.float32)
    in_maps = []
    for k in range(N_CORES):
        sl = slice(k * BC, (k + 1) * BC)
        in_maps.append(
            {
                "rt": _to_component_major(states_real[sl]),
                "mt": _to_component_major(states_imag[sl]),
                "cst": cst,
            }
        )
    nc = _get_program()
    res = bass_utils.run_bass_kernel_spmd(
        nc, in_maps, core_ids=list(range(N_CORES)), **hw_kwargs
    )
    out = np.concatenate(
        [_from_out32(res.results[k]["out"]) for k in range(N_CORES)]
    ).astype(np.float32)
    return out, res


def kernel(ry_params, rz_params, states_real, states_imag):
    out, _ = _run(ry_params, rz_params, states_real, states_imag)
    return out
